# revision 1
# baseline (speedup 1.0000x reference)
"""MoE grouped-linear (ragged matmul + bias) on 8 TRN2 NeuronCores.

Expert-parallel sharding: core e computes tokens of expert e:
    out_e = X_e[cap, 2048] @ W_e[2048, 8192] + bias
Tokens are pre-sorted by expert (contiguous groups), so the "all-to-all"
is a free host-side slice/concat. No on-device collectives.

Per-core kernel (_build2, the production path): bf16 matmuls (rel err
2.4e-3 on this problem, gate is 2e-2), loop order (n-block of 4, mi, k,
ni-in-block) so each stationary x^T[mi,k] serves 4 consecutive 512-col
matmuls — this amortizes/hides LDWEIGHTS, which at ~107 ns per 128-col
load is NOT hidden when the stationary changes every matmul (the old
fp32r kernel's ~258 ns/MM vs the 213.3 ns = 512row/2.4GHz warm floor).
All 8 PSUM banks cycle through the 4 concurrent accumulation chains;
bias is fused into the PSUM eviction on the vector engine; W streams
once in [128,16,2048] blocks with per-k-slice DMAs; X^T + bias load on
the gpsimd queue.

Measured via reps-slope R=1 vs R=33 (no NTFF under axon): ~435-440
us/core in quiet windows = the 2048x213.3ns tensor-engine floor at the
full 2.4 GHz clock (same-stationary microbench hits 219 ns/MM, proving
the clock and that LDW was the old kernel's gap). Under sustained load
or tenant/thermal contention the package duty-cycles the PE clock
(K=4/8 HAM states -> ~1.8-2.0 GHz effective) and ALL kernel variants
(incl. no-evict / same-stationary) converge to ~540-585 us — that
regime is power-bound, not structure-bound. fp8 e4m3 DoubleRow (the
only >1x mode) fails accuracy: 4.0e-2 single-pass, 2.7e-2 even with a
2-pass hi/lo split of either operand (e4m3's ~2.5% per-element error
x sqrt(K=2048) accumulation); 3-pass passes but is slower than bf16.
nt=1024 moving is rejected by the walrus birverifier (512 max).
"""

import numpy as np

E, IN, OUT = 8, 2048, 8192
P = 128
NTILE = 512

_cache = {}


def _build(cap, dtype_name="float32r", reps=1, mode="full", ntile=None):
    import contextlib

    import concourse.mybir as mybir
    import concourse.tile as tile
    from concourse import bacc

    mm_dt = getattr(mybir.dt, dtype_name)
    nt = ntile or NTILE
    KT = IN // P            # 16 k-tiles
    MT = cap // P           # m-tiles per core
    NT = OUT // nt          # n-tiles

    nc = bacc.Bacc(None, target_bir_lowering=False, debug=False)
    with tile.TileContext(nc) as tc:
        with tc.tile_pool(name="dram", bufs=1, space="DRAM") as dram:
            # xt_d[mi, p, k, j] = X[mi*P + j, k*P + p] — per-mi contiguous
            # 1MB slices so the first matmul group can start after ~1MB of DMA
            xt_d = dram.tile((MT, P, KT, P), mm_dt, kind="ExternalInput")
            w_d = dram.tile((P, KT, OUT), mm_dt, kind="ExternalInput")
            bias_d = dram.tile((P, OUT), mybir.dt.float32, kind="ExternalInput")
            out_d = dram.tile((P, MT, OUT), mybir.dt.float32, kind="ExternalOutput")

            with tc.tile_pool(name="resident", bufs=1) as res_pool, \
                 tc.tile_pool(name="wchunk", bufs=2) as w_pool, \
                 tc.tile_pool(name="evict", bufs=6) as o_pool, \
                 tc.tile_pool(name="acc", bufs=(3 if nt > 512 else 6), space="PSUM") as ps_pool:
                loop = tc.For_i(0, reps, 1) if reps > 1 else contextlib.nullcontext()
                with loop:
                    # W stream owns the sync queue; X^T + bias load in
                    # parallel on the gpsimd queue, first-needed first.
                    w_sbs = [None] * NT
                    w_sbs[0] = w_pool.tile([P, KT, nt], mm_dt, tag="w",
                                           name="w_sb0")
                    nc.sync.dma_start(w_sbs[0][:], w_d[:, :, 0:nt])

                    xt_sb = [res_pool.tile([P, KT, P], mm_dt, tag=f"xt{mi}",
                                           name=f"xt_sb{mi}")
                             for mi in range(MT)]
                    bias_sb = res_pool.tile([P, OUT], mybir.dt.float32)
                    nc.gpsimd.dma_start(xt_sb[0][:], xt_d[0])
                    nc.gpsimd.dma_start(bias_sb[:], bias_d[:])
                    for mi in range(1, MT):
                        nc.gpsimd.dma_start(xt_sb[mi][:], xt_d[mi])

                    for ni in range(NT):
                        w_sb = w_sbs[ni]
                        if w_sb is None and mode in ("mm_only", "same_w"):
                            w_sb = w_sbs[0]
                        elif w_sb is None:
                            w_sb = w_pool.tile([P, KT, nt], mm_dt, tag="w",
                                               name=f"w_sb{ni}")
                            nc.sync.dma_start(
                                w_sb[:], w_d[:, :, ni * nt:(ni + 1) * nt])
                        for mi in range(MT):
                            ps = ps_pool.tile([P, nt], mybir.dt.float32)
                            for k in range(KT):
                                nc.tensor.matmul(
                                    ps[:],
                                    lhsT=xt_sb[0][:, 0, :] if mode == "same_w"
                                    else xt_sb[mi][:, k, :],
                                    rhs=w_sb[:, k, :],
                                    start=(k == 0),
                                    stop=(k == KT - 1),
                                )
                            if mode in ("mm_only", "same_w") and not (ni == NT - 1 and mi == MT - 1):
                                continue
                            o_sb = o_pool.tile([P, nt], mybir.dt.float32)
                            nc.vector.tensor_add(
                                out=o_sb[:], in0=ps[:],
                                in1=bias_sb[:, ni * nt:(ni + 1) * nt])
                            nc.sync.dma_start(
                                out_d[:, mi, ni * nt:(ni + 1) * nt], o_sb[:])
    nc.compile()
    names = dict(xt=xt_d.name, w=w_d.name, bias=bias_d.name, out=out_d.name)
    return nc, names


def _get(cap, dtype_name="float32r", reps=1, mode="full", ntile=None):
    key = (cap, dtype_name, reps, mode, ntile)
    if key not in _cache:
        _cache[key] = _build(cap, dtype_name, reps, mode, ntile)
    return _cache[key]


def _build2(cap, dtype_name="bfloat16", reps=1, nblk=4, psum_bufs=8,
            mode="full"):
    """LDW-amortized variant: loop (ni_blk, mi, k, ni-in-blk) so each
    stationary x^T[mi,k] serves `nblk` consecutive 512-col matmuls.
    W is streamed once, in [128, KT, nblk*512] blocks, per-k-slice DMAs.
    mode: full | no_evict (only last gen evicts) | same_w (fixed stationary)
    """
    import contextlib

    import concourse.mybir as mybir
    import concourse.tile as tile
    from concourse import bacc

    mm_dt = getattr(mybir.dt, dtype_name)
    nt = NTILE                      # 512
    KT = IN // P                    # 16
    MT = cap // P                   # m-tiles
    NBLK = OUT // (nblk * nt)       # blocks of nblk n-tiles
    bw = nblk * nt                  # block width in cols

    nc = bacc.Bacc(None, target_bir_lowering=False, debug=False)
    with tile.TileContext(nc) as tc:
        with tc.tile_pool(name="dram", bufs=1, space="DRAM") as dram:
            xt_d = dram.tile((MT, P, KT, P), mm_dt, kind="ExternalInput")
            w_d = dram.tile((P, KT, OUT), mm_dt, kind="ExternalInput")
            bias_d = dram.tile((P, OUT), mybir.dt.float32, kind="ExternalInput")
            out_d = dram.tile((P, MT, OUT), mybir.dt.float32, kind="ExternalOutput")

            with tc.tile_pool(name="resident", bufs=1) as res_pool, \
                 tc.tile_pool(name="wblk", bufs=2) as w_pool, \
                 tc.tile_pool(name="evict", bufs=4) as o_pool, \
                 tc.tile_pool(name="acc", bufs=psum_bufs, space="PSUM") as ps_pool:
                loop = tc.For_i(0, reps, 1) if reps > 1 else contextlib.nullcontext()
                with loop:
                    w_sbs = [None] * NBLK
                    w_sbs[0] = w_pool.tile([P, KT, bw], mm_dt, tag="w",
                                           name="w_sb0")
                    # per-k-slice DMAs so the first matmul is gated on
                    # one k-slice, not the whole 8MB block
                    for k in range(KT):
                        nc.sync.dma_start(w_sbs[0][:, k], w_d[:, k, 0:bw])

                    xt_sb = [res_pool.tile([P, KT, P], mm_dt, tag=f"xt{mi}",
                                           name=f"xt_sb{mi}")
                             for mi in range(MT)]
                    bias_sb = res_pool.tile([P, OUT], mybir.dt.float32)
                    nc.gpsimd.dma_start(xt_sb[0][:], xt_d[0])
                    nc.gpsimd.dma_start(bias_sb[:], bias_d[:])
                    for mi in range(1, MT):
                        nc.gpsimd.dma_start(xt_sb[mi][:], xt_d[mi])

                    for blk in range(NBLK):
                        w_sb = w_sbs[blk]
                        if w_sb is None:
                            w_sb = w_pool.tile([P, KT, bw], mm_dt, tag="w",
                                               name=f"w_sb{blk}")
                            for k in range(KT):
                                nc.sync.dma_start(
                                    w_sb[:, k],
                                    w_d[:, k, blk * bw:(blk + 1) * bw])
                        for mi in range(MT):
                            pss = [ps_pool.tile([P, nt], mybir.dt.float32,
                                                tag="ps",
                                                name=f"ps{blk}_{mi}_{j}")
                                   for j in range(nblk)]
                            for k in range(KT):
                                for ni in range(nblk):
                                    nc.tensor.matmul(
                                        pss[ni][:],
                                        lhsT=xt_sb[0][:, 0, :] if mode == "same_w"
                                        else xt_sb[mi][:, k, :],
                                        rhs=w_sb[:, k, ni * nt:(ni + 1) * nt],
                                        start=(k == 0),
                                        stop=(k == KT - 1),
                                    )
                            if mode in ("no_evict", "same_w") and not (
                                    blk == NBLK - 1 and mi == MT - 1):
                                continue
                            for ni in range(nblk):
                                o_sb = o_pool.tile([P, nt], mybir.dt.float32)
                                col0 = blk * bw + ni * nt
                                nc.vector.tensor_add(
                                    out=o_sb[:], in0=pss[ni][:],
                                    in1=bias_sb[:, col0:col0 + nt])
                                nc.sync.dma_start(
                                    out_d[:, mi, col0:col0 + nt], o_sb[:])
    nc.compile()
    names = dict(xt=xt_d.name, w=w_d.name, bias=bias_d.name, out=out_d.name)
    return nc, names


def _get2(cap, dtype_name="bfloat16", reps=1, nblk=4, psum_bufs=8,
          mode="full"):
    key = ("v2", cap, dtype_name, reps, nblk, psum_bufs, mode)
    if key not in _cache:
        _cache[key] = _build2(cap, dtype_name, reps, nblk, psum_bufs, mode)
    return _cache[key]


def kernel(inputs, weight, group_sizes, bias):
    import ml_dtypes

    from concourse.bass_utils import run_bass_kernel_spmd

    M = inputs.shape[0]
    gs = np.asarray(group_sizes, dtype=np.int64)
    # per-token expert id exactly as the reference's jnp.repeat(...,
    # total_repeat_length=M): truncate or pad with the last expert id
    ids = np.repeat(np.arange(E), gs)
    ids = ids[:M] if len(ids) >= M else np.concatenate(
        [ids, np.full(M - len(ids), E - 1)])
    counts = np.bincount(ids, minlength=E)
    starts = np.concatenate([[0], np.cumsum(counts)])[:E]

    cap = max(P, int(-(-counts.max() // P) * P))
    nc, names = _get2(cap)

    x = np.asarray(inputs, dtype=np.float32).astype(ml_dtypes.bfloat16)
    w = np.asarray(weight, dtype=np.float32).astype(ml_dtypes.bfloat16)
    bias_rep = np.ascontiguousarray(
        np.broadcast_to(np.asarray(bias, np.float32), (P, OUT)))

    in_maps = []
    for e in range(E):
        xe = x[starts[e]:starts[e] + counts[e]]
        if xe.shape[0] < cap:
            xe = np.concatenate(
                [xe, np.zeros((cap - xe.shape[0], IN), ml_dtypes.bfloat16)])
        # [cap, IN] -> (MT, P, KT, P): xt[mi, p, k, j] = X[mi*P+j, k*P+p]
        xt = np.ascontiguousarray(
            xe.reshape(cap // P, P, IN // P, P).transpose(0, 3, 2, 1))
        # [IN, OUT] -> (P, KT, OUT): wt[p, a, n] = W[a*P+p, n]
        we = np.ascontiguousarray(
            w[e].reshape(IN // P, P, OUT).transpose(1, 0, 2))
        in_maps.append({names["xt"]: xt, names["w"]: we,
                        names["bias"]: bias_rep})

    res = run_bass_kernel_spmd(nc, in_maps, core_ids=list(range(E)))
    out = np.empty((M, OUT), dtype=np.float32)
    for e in range(E):
        oe = res.results[e][names["out"]]          # (P, cap//P, OUT)
        oe = oe.transpose(1, 0, 2).reshape(cap, OUT)
        out[starts[e]:starts[e] + counts[e]] = oe[:counts[e]]
    return out



# revision 12
# speedup vs baseline: 1.1172x; 1.1172x over previous
"""MoE grouped-linear (ragged matmul + bias) on 8 TRN2 NeuronCores.

Expert-parallel sharding: core e computes tokens of expert e:
    out_e = X_e[cap, 2048] @ W_e[2048, 8192] + bias
Tokens are pre-sorted by expert (contiguous groups), so the "all-to-all"
is a free host-side slice/concat. No on-device collectives.

Per-core kernel (_build2, the production path): bf16 matmuls (rel err
2.4e-3 on this problem, gate is 2e-2), loop order (n-block of 4, mi, k,
ni-in-block) so each stationary x^T[mi,k] serves 4 consecutive 512-col
matmuls — this amortizes/hides LDWEIGHTS, which at ~107 ns per 128-col
load is NOT hidden when the stationary changes every matmul (the old
fp32r kernel's ~258 ns/MM vs the 213.3 ns = 512row/2.4GHz warm floor).
All 8 PSUM banks cycle through the 4 concurrent accumulation chains;
bias is fused into the PSUM eviction on the vector engine; W streams
once in [128,16,2048] blocks with per-k-slice DMAs; X^T + bias load on
the gpsimd queue.

Measured via reps-slope R=1 vs R=33 (no NTFF under axon): ~435-440
us/core in quiet windows = the 2048x213.3ns tensor-engine floor at the
full 2.4 GHz clock (same-stationary microbench hits 219 ns/MM, proving
the clock and that LDW was the old kernel's gap). Under sustained load
or tenant/thermal contention the package duty-cycles the PE clock
(K=4/8 HAM states -> ~1.8-2.0 GHz effective) and ALL kernel variants
(incl. no-evict / same-stationary) converge to ~540-585 us — that
regime is power-bound, not structure-bound. fp8 e4m3 DoubleRow (the
only >1x mode) fails accuracy: 4.0e-2 single-pass, 2.7e-2 even with a
2-pass hi/lo split of either operand (e4m3's ~2.5% per-element error
x sqrt(K=2048) accumulation); 3-pass passes but is slower than bf16.
nt=1024 moving is rejected by the walrus birverifier (512 max).
"""

import numpy as np

E, IN, OUT = 8, 2048, 8192
P = 128
NTILE = 512

_cache = {}


def _build(cap, dtype_name="float32r", reps=1, mode="full", ntile=None):
    import contextlib

    import concourse.mybir as mybir
    import concourse.tile as tile
    from concourse import bacc

    mm_dt = getattr(mybir.dt, dtype_name)
    nt = ntile or NTILE
    KT = IN // P            # 16 k-tiles
    MT = cap // P           # m-tiles per core
    NT = OUT // nt          # n-tiles

    nc = bacc.Bacc(None, target_bir_lowering=False, debug=False)
    with tile.TileContext(nc) as tc:
        with tc.tile_pool(name="dram", bufs=1, space="DRAM") as dram:
            # xt_d[mi, p, k, j] = X[mi*P + j, k*P + p] — per-mi contiguous
            # 1MB slices so the first matmul group can start after ~1MB of DMA
            xt_d = dram.tile((MT, P, KT, P), mm_dt, kind="ExternalInput")
            w_d = dram.tile((P, KT, OUT), mm_dt, kind="ExternalInput")
            bias_d = dram.tile((P, OUT), mybir.dt.float32, kind="ExternalInput")
            out_d = dram.tile((P, MT, OUT), mybir.dt.float32, kind="ExternalOutput")

            with tc.tile_pool(name="resident", bufs=1) as res_pool, \
                 tc.tile_pool(name="wchunk", bufs=2) as w_pool, \
                 tc.tile_pool(name="evict", bufs=6) as o_pool, \
                 tc.tile_pool(name="acc", bufs=(3 if nt > 512 else 6), space="PSUM") as ps_pool:
                loop = tc.For_i(0, reps, 1) if reps > 1 else contextlib.nullcontext()
                with loop:
                    # W stream owns the sync queue; X^T + bias load in
                    # parallel on the gpsimd queue, first-needed first.
                    w_sbs = [None] * NT
                    w_sbs[0] = w_pool.tile([P, KT, nt], mm_dt, tag="w",
                                           name="w_sb0")
                    nc.sync.dma_start(w_sbs[0][:], w_d[:, :, 0:nt])

                    xt_sb = [res_pool.tile([P, KT, P], mm_dt, tag=f"xt{mi}",
                                           name=f"xt_sb{mi}")
                             for mi in range(MT)]
                    bias_sb = res_pool.tile([P, OUT], mybir.dt.float32)
                    nc.gpsimd.dma_start(xt_sb[0][:], xt_d[0])
                    nc.gpsimd.dma_start(bias_sb[:], bias_d[:])
                    for mi in range(1, MT):
                        nc.gpsimd.dma_start(xt_sb[mi][:], xt_d[mi])

                    for ni in range(NT):
                        w_sb = w_sbs[ni]
                        if w_sb is None and mode in ("mm_only", "same_w"):
                            w_sb = w_sbs[0]
                        elif w_sb is None:
                            w_sb = w_pool.tile([P, KT, nt], mm_dt, tag="w",
                                               name=f"w_sb{ni}")
                            nc.sync.dma_start(
                                w_sb[:], w_d[:, :, ni * nt:(ni + 1) * nt])
                        for mi in range(MT):
                            ps = ps_pool.tile([P, nt], mybir.dt.float32)
                            for k in range(KT):
                                nc.tensor.matmul(
                                    ps[:],
                                    lhsT=xt_sb[0][:, 0, :] if mode == "same_w"
                                    else xt_sb[mi][:, k, :],
                                    rhs=w_sb[:, k, :],
                                    start=(k == 0),
                                    stop=(k == KT - 1),
                                )
                            if mode in ("mm_only", "same_w") and not (ni == NT - 1 and mi == MT - 1):
                                continue
                            o_sb = o_pool.tile([P, nt], mybir.dt.float32)
                            nc.vector.tensor_add(
                                out=o_sb[:], in0=ps[:],
                                in1=bias_sb[:, ni * nt:(ni + 1) * nt])
                            nc.sync.dma_start(
                                out_d[:, mi, ni * nt:(ni + 1) * nt], o_sb[:])
    nc.compile()
    names = dict(xt=xt_d.name, w=w_d.name, bias=bias_d.name, out=out_d.name)
    return nc, names


def _get(cap, dtype_name="float32r", reps=1, mode="full", ntile=None):
    key = (cap, dtype_name, reps, mode, ntile)
    if key not in _cache:
        _cache[key] = _build(cap, dtype_name, reps, mode, ntile)
    return _cache[key]


def _build2(cap, dtype_name="bfloat16", reps=1, nblk=4, psum_bufs=8,
            mode="full"):
    """LDW-amortized variant: loop (ni_blk, mi, k, ni-in-blk) so each
    stationary x^T[mi,k] serves `nblk` consecutive 512-col matmuls.
    W is streamed once, in [128, KT, nblk*512] blocks, per-k-slice DMAs.
    mode: full | no_evict (only last gen evicts) | same_w (fixed stationary)
    """
    import contextlib

    import concourse.mybir as mybir
    import concourse.tile as tile
    from concourse import bacc

    mm_dt = getattr(mybir.dt, dtype_name)
    nt = NTILE                      # 512
    KT = IN // P                    # 16
    MT = cap // P                   # m-tiles
    NBLK = OUT // (nblk * nt)       # blocks of nblk n-tiles
    bw = nblk * nt                  # block width in cols

    nc = bacc.Bacc(None, target_bir_lowering=False, debug=False)
    with tile.TileContext(nc) as tc:
        with tc.tile_pool(name="dram", bufs=1, space="DRAM") as dram:
            xt_d = dram.tile((MT, P, KT, P), mm_dt, kind="ExternalInput")
            w_d = dram.tile((P, KT, OUT), mm_dt, kind="ExternalInput")
            bias_d = dram.tile((P, OUT), mybir.dt.float32, kind="ExternalInput")
            out_d = dram.tile((P, MT, OUT), mybir.dt.float32, kind="ExternalOutput")

            with tc.tile_pool(name="resident", bufs=1) as res_pool, \
                 tc.tile_pool(name="wblk", bufs=2) as w_pool, \
                 tc.tile_pool(name="evict", bufs=8) as o_pool, \
                 tc.tile_pool(name="acc", bufs=psum_bufs, space="PSUM") as ps_pool:
                loop = tc.For_i(0, reps, 1) if reps > 1 else contextlib.nullcontext()
                with loop:
                    w_sbs = [None] * NBLK
                    w_sbs[0] = w_pool.tile([P, KT, bw], mm_dt, tag="w",
                                           name="w_sb0")
                    # per-k-slice DMAs so the first matmul is gated on
                    # one k-slice, not the whole 8MB block
                    for k in range(KT):
                        nc.sync.dma_start(w_sbs[0][:, k], w_d[:, k, 0:bw])

                    xt_sb = [res_pool.tile([P, KT, P], mm_dt, tag=f"xt{mi}",
                                           name=f"xt_sb{mi}")
                             for mi in range(MT)]
                    bias_sb = res_pool.tile([P, OUT], mybir.dt.float32)
                    nc.gpsimd.dma_start(xt_sb[0][:], xt_d[0])
                    nc.gpsimd.dma_start(bias_sb[:], bias_d[:])
                    for mi in range(1, MT):
                        nc.gpsimd.dma_start(xt_sb[mi][:], xt_d[mi])

                    for blk in range(NBLK):
                        w_sb = w_sbs[blk]
                        if w_sb is None:
                            w_sb = w_pool.tile([P, KT, bw], mm_dt, tag="w",
                                               name=f"w_sb{blk}")
                            for k in range(KT):
                                nc.sync.dma_start(
                                    w_sb[:, k],
                                    w_d[:, k, blk * bw:(blk + 1) * bw])
                        for mi in range(MT):
                            pss = [ps_pool.tile([P, nt], mybir.dt.float32,
                                                tag="ps",
                                                name=f"ps{blk}_{mi}_{j}")
                                   for j in range(nblk)]
                            for k in range(KT):
                                for ni in range(nblk):
                                    nc.tensor.matmul(
                                        pss[ni][:],
                                        lhsT=xt_sb[0][:, 0, :] if mode == "same_w"
                                        else xt_sb[mi][:, k, :],
                                        rhs=w_sb[:, k, ni * nt:(ni + 1) * nt],
                                        start=(k == 0),
                                        stop=(k == KT - 1),
                                    )
                            if mode in ("no_evict", "same_w") and not (
                                    blk == NBLK - 1 and mi == MT - 1):
                                continue
                            for ni in range(nblk):
                                o_sb = o_pool.tile([P, nt], mybir.dt.float32)
                                col0 = blk * bw + ni * nt
                                nc.vector.tensor_add(
                                    out=o_sb[:], in0=pss[ni][:],
                                    in1=bias_sb[:, col0:col0 + nt])
                                nc.sync.dma_start(
                                    out_d[:, mi, col0:col0 + nt], o_sb[:])
    nc.compile()
    names = dict(xt=xt_d.name, w=w_d.name, bias=bias_d.name, out=out_d.name)
    return nc, names


def _get2(cap, dtype_name="bfloat16", reps=1, nblk=4, psum_bufs=8,
          mode="full"):
    key = ("v2", cap, dtype_name, reps, nblk, psum_bufs, mode)
    if key not in _cache:
        _cache[key] = _build2(cap, dtype_name, reps, nblk, psum_bufs, mode)
    return _cache[key]


def _build3(cap, reps=1, out_dt_name="bfloat16"):
    """Cold-start-optimized variant.

    Block A (first 2048 cols): k-outer over mi-pairs so the first sweep
    consumes W k-slices at ~300GB/s (supply ~350) instead of mi-outer's
    600GB/s — kills the block-0 DMA stall. G=4 stationary reuse kept
    (each xt[mi,k] serves ni0..3); 2mi x 4ni = 8 live PSUM banks.
    Blocks B1..B3 (cols 2048..8191): mi-outer as _build2 (prefetched).
    Out DMAs ride the Activation queue so they never block the SP
    queue's W prefetch; non-critical loads (xt2b/3b, xt4..7, biasB) are
    emitted on the Act queue BETWEEN eviction DMAs, so the out-DMAs'
    semaphore waits throttle them until the cold-start window is over.
    bias is bf16 [P, OUT]; outputs are stored bf16 (abs err +<=0.011 vs
    gate 0.114). First matmul gated on ~190KB: W k0 in 4 chunks, xt0/1
    split at k0..1. Tail: last mi's 4 out-DMAs split across SP + Act.
    """
    import contextlib

    import concourse.mybir as mybir
    import concourse.tile as tile
    from concourse import bacc

    mm_dt = mybir.dt.bfloat16
    out_dt = getattr(mybir.dt, out_dt_name)
    nt = NTILE                      # 512
    KT = IN // P                    # 16
    MT = cap // P                   # m-tiles
    nblk = 4
    bw = nblk * nt                  # 2048
    NBLK = OUT // bw                # 4 (A + 3 B-blocks)
    GM = min(2, MT)                 # mi-group size in block A
    tuned = MT == 8                 # DMA schedule tuned for cap=1024

    nc = bacc.Bacc(None, target_bir_lowering=False, debug=False)
    with tile.TileContext(nc) as tc:
        with tc.tile_pool(name="dram", bufs=1, space="DRAM") as dram:
            xt_d = dram.tile((MT, P, KT, P), mm_dt, kind="ExternalInput")
            w_d = dram.tile((P, KT, OUT), mm_dt, kind="ExternalInput")
            bias_d = dram.tile((P, OUT), mm_dt, kind="ExternalInput")
            out_d = dram.tile((P, MT, OUT), out_dt, kind="ExternalOutput")

            with tc.tile_pool(name="resident", bufs=1) as res_pool, \
                 tc.tile_pool(name="wblk", bufs=2) as w_pool, \
                 tc.tile_pool(name="evict", bufs=8) as o_pool, \
                 tc.tile_pool(name="acc", bufs=8, space="PSUM") as ps_pool:
                loop = tc.For_i(0, reps, 1) if reps > 1 else contextlib.nullcontext()
                with loop:
                    # ---- block A weights: per-k DMAs, k0 chunked x4,
                    # k1 chunked x2 so the first matmuls gate on 128KB
                    wA = w_pool.tile([P, KT, bw], mm_dt, tag="w", name="wA")
                    for c in range(4):
                        nc.sync.dma_start(wA[:, 0, c * nt:(c + 1) * nt],
                                          w_d[:, 0, c * nt:(c + 1) * nt])
                    for c in range(2):
                        nc.sync.dma_start(
                            wA[:, 1, c * 2 * nt:(c + 1) * 2 * nt],
                            w_d[:, 1, c * 2 * nt:(c + 1) * 2 * nt])
                    for k in range(2, KT):
                        nc.sync.dma_start(wA[:, k], w_d[:, k, 0:bw])

                    # ---- early-critical loads on the gpsimd queue
                    xt_sb = [res_pool.tile([P, KT, P], mm_dt, tag=f"xt{mi}",
                                           name=f"xt_sb{mi}")
                             for mi in range(MT)]
                    bias_sb = res_pool.tile([P, OUT], mm_dt)
                    if tuned:
                        for mi in (0, 1):
                            nc.gpsimd.dma_start(xt_sb[mi][:, 0:2],
                                                xt_d[mi][:, 0:2])
                        for mi in (0, 1):
                            nc.gpsimd.dma_start(xt_sb[mi][:, 2:KT],
                                                xt_d[mi][:, 2:KT])
                        nc.gpsimd.dma_start(bias_sb[:, 0:bw], bias_d[:, 0:bw])
                        # later-needed xt tiles ride the SP queue AFTER
                        # block A's W slices: FIFO keeps them off the
                        # cold-start window; B1's prefetch has 80us slack
                        for mi in range(2, MT):
                            nc.sync.dma_start(xt_sb[mi][:], xt_d[mi])
                    else:
                        for mi in range(MT):
                            nc.gpsimd.dma_start(xt_sb[mi][:], xt_d[mi])
                        nc.gpsimd.dma_start(bias_sb[:], bias_d[:])

                    def evict(ps, mi, col0, dma_eng=None):
                        o_sb = o_pool.tile([P, nt], out_dt)
                        nc.vector.tensor_add(
                            out=o_sb[:], in0=ps[:],
                            in1=bias_sb[:, col0:col0 + nt])
                        (dma_eng or nc.scalar).dma_start(
                            out_d[:, mi, col0:col0 + nt], o_sb[:])

                    # ---- block A: k-outer over mi-groups
                    for g in range(0, MT, GM):
                        gm = min(GM, MT - g)
                        pss = [ps_pool.tile([P, nt], mybir.dt.float32,
                                            tag="ps", name=f"psA{g}_{j}")
                               for j in range(gm * nblk)]
                        for k in range(KT):
                            for mj in range(gm):
                                for ni in range(nblk):
                                    nc.tensor.matmul(
                                        pss[mj * nblk + ni][:],
                                        lhsT=xt_sb[g + mj][:, k, :],
                                        rhs=wA[:, k, ni * nt:(ni + 1) * nt],
                                        start=(k == 0),
                                        stop=(k == KT - 1),
                                    )
                        for mj in range(gm):
                            for ni in range(nblk):
                                evict(pss[mj * nblk + ni], g + mj, ni * nt)
                        # throttled loads: queued on Act behind this group's
                        # out-DMAs, so they transfer only after the cold
                        # window; each arrives well before it is needed


                    # ---- blocks B1..B3: mi-outer (W prefetched)
                    for blk in range(1, NBLK):
                        w_sb = w_pool.tile([P, KT, bw], mm_dt, tag="w",
                                           name=f"wB{blk}")
                        for k in range(KT):
                            nc.sync.dma_start(
                                w_sb[:, k], w_d[:, k, blk * bw:(blk + 1) * bw])
                        if tuned:
                            nc.sync.dma_start(
                                bias_sb[:, blk * bw:(blk + 1) * bw],
                                bias_d[:, blk * bw:(blk + 1) * bw])
                        for mi in range(MT):
                            pss = [ps_pool.tile([P, nt], mybir.dt.float32,
                                                tag="ps",
                                                name=f"ps{blk}_{mi}_{j}")
                                   for j in range(nblk)]
                            for k in range(KT):
                                for ni in range(nblk):
                                    nc.tensor.matmul(
                                        pss[ni][:],
                                        lhsT=xt_sb[mi][:, k, :],
                                        rhs=w_sb[:, k, ni * nt:(ni + 1) * nt],
                                        start=(k == 0),
                                        stop=(k == KT - 1),
                                    )
                            last = blk == NBLK - 1 and mi == MT - 1
                            for ni in range(nblk):
                                dq = nc.sync if (last and ni < 2) else None
                                evict(pss[ni], mi, blk * bw + ni * nt,
                                      dma_eng=dq)
    nc.compile()
    names = dict(xt=xt_d.name, w=w_d.name, bias=bias_d.name, out=out_d.name)
    return nc, names


def _build4(cap, reps=1, out_dt_name="bfloat16"):
    """A/B/C-segment variant (requires LDW hidden at G=2, measured on HW).

    A: cols 0..1023, k-outer, GM=4 mi-group, nblk=2 (G=2): W demand
    ~150GB/s in the cold window, 8.25MB of early DMA vs ~9.6MB capacity.
    B1..B3: cols 1024..7167, bw=2048 nblk=4 mi-outer (prefetched).
    C: cols 7168..8191, bw=1024 nblk=2 mi-outer: last mi drains only 2
    chains -> short tail; its 2 out-DMAs split across SP/Act queues.
    """
    import contextlib

    import concourse.mybir as mybir
    import concourse.tile as tile
    from concourse import bacc

    mm_dt = mybir.dt.bfloat16
    out_dt = getattr(mybir.dt, out_dt_name)
    nt = NTILE                      # 512
    KT = IN // P                    # 16
    MT = cap // P
    tuned = MT == 8

    nc = bacc.Bacc(None, target_bir_lowering=False, debug=False)
    with tile.TileContext(nc) as tc:
        with tc.tile_pool(name="dram", bufs=1, space="DRAM") as dram:
            xt_d = dram.tile((MT, P, KT, P), mm_dt, kind="ExternalInput")
            w_d = dram.tile((P, KT, OUT), mm_dt, kind="ExternalInput")
            bias_d = dram.tile((P, OUT), mm_dt, kind="ExternalInput")
            out_d = dram.tile((P, MT, OUT), out_dt, kind="ExternalOutput")

            with tc.tile_pool(name="resident", bufs=1) as res_pool, \
                 tc.tile_pool(name="wblk", bufs=2) as w_pool, \
                 tc.tile_pool(name="evict", bufs=8) as o_pool, \
                 tc.tile_pool(name="acc", bufs=8, space="PSUM") as ps_pool:
                loop = tc.For_i(0, reps, 1) if reps > 1 else contextlib.nullcontext()
                with loop:
                    # ---- segment A weights: [P, KT, 1024], per-k DMAs,
                    # k0 in two 512-col chunks
                    awb = 2 * nt    # 1024
                    wA = w_pool.tile([P, KT, awb], mm_dt, tag="w", name="wA")
                    for c in range(2):
                        nc.sync.dma_start(wA[:, 0, c * nt:(c + 1) * nt],
                                          w_d[:, 0, c * nt:(c + 1) * nt])
                    for k in range(1, KT):
                        nc.sync.dma_start(wA[:, k], w_d[:, k, 0:awb])

                    xt_sb = [res_pool.tile([P, KT, P], mm_dt, tag=f"xt{mi}",
                                           name=f"xt_sb{mi}")
                             for mi in range(MT)]
                    bias_sb = res_pool.tile([P, OUT], mm_dt)
                    if tuned:
                        # group0 = mi0..3: k0..3 chunks first, then k4..15,
                        # then xt4..7 + biasA (needed from 27.3us)
                        for mi in range(4):
                            nc.gpsimd.dma_start(xt_sb[mi][:, 0:4],
                                                xt_d[mi][:, 0:4])
                        for mi in range(4):
                            nc.gpsimd.dma_start(xt_sb[mi][:, 4:KT],
                                                xt_d[mi][:, 4:KT])
                        for mi in (4, 5):
                            nc.gpsimd.dma_start(xt_sb[mi][:], xt_d[mi])
                        nc.gpsimd.dma_start(bias_sb[:, 0:awb],
                                            bias_d[:, 0:awb])
                        for mi in (6, 7):
                            nc.gpsimd.dma_start(xt_sb[mi][:], xt_d[mi])
                        # remaining bias loads ride SP after each B/C
                        # block's W slices (FIFO keeps them off the cold
                        # window; no semaphore-waits involved)
                    else:
                        for mi in range(MT):
                            nc.gpsimd.dma_start(xt_sb[mi][:], xt_d[mi])
                        nc.gpsimd.dma_start(bias_sb[:], bias_d[:])

                    def evict(ps, mi, col0, dma_eng=None):
                        o_sb = o_pool.tile([P, nt], out_dt)
                        nc.vector.tensor_add(
                            out=o_sb[:], in0=ps[:],
                            in1=bias_sb[:, col0:col0 + nt])
                        (dma_eng or nc.scalar).dma_start(
                            out_d[:, mi, col0:col0 + nt], o_sb[:])

                    # ---- segment A: k-outer, groups of GM=4 mi, nblk=2
                    GM = min(4, MT)
                    for g in range(0, MT, GM):
                        gm = min(GM, MT - g)
                        pss = [ps_pool.tile([P, nt], mybir.dt.float32,
                                            tag="ps", name=f"psA{g}_{j}")
                               for j in range(gm * 2)]
                        for k in range(KT):
                            for mj in range(gm):
                                for ni in range(2):
                                    nc.tensor.matmul(
                                        pss[mj * 2 + ni][:],
                                        lhsT=xt_sb[g + mj][:, k, :],
                                        rhs=wA[:, k, ni * nt:(ni + 1) * nt],
                                        start=(k == 0),
                                        stop=(k == KT - 1),
                                    )
                        for mj in range(gm):
                            for ni in range(2):
                                evict(pss[mj * 2 + ni], g + mj, ni * nt)

                    # ---- segments B: bw=2048, nblk=4, mi-outer
                    nblk = 4
                    bw = nblk * nt
                    nB = (OUT - 2 * awb) // bw      # 3
                    for blk in range(nB):
                        col_b = awb + blk * bw
                        w_sb = w_pool.tile([P, KT, bw], mm_dt, tag="w",
                                           name=f"wB{blk}")
                        for k in range(KT):
                            nc.sync.dma_start(
                                w_sb[:, k], w_d[:, k, col_b:col_b + bw])
                        if tuned:
                            nc.sync.dma_start(bias_sb[:, col_b:col_b + bw],
                                              bias_d[:, col_b:col_b + bw])
                        for mi in range(MT):
                            pss = [ps_pool.tile([P, nt], mybir.dt.float32,
                                                tag="ps",
                                                name=f"ps{blk}_{mi}_{j}")
                                   for j in range(nblk)]
                            for k in range(KT):
                                for ni in range(nblk):
                                    nc.tensor.matmul(
                                        pss[ni][:],
                                        lhsT=xt_sb[mi][:, k, :],
                                        rhs=w_sb[:, k, ni * nt:(ni + 1) * nt],
                                        start=(k == 0),
                                        stop=(k == KT - 1),
                                    )
                            for ni in range(nblk):
                                evict(pss[ni], mi, col_b + ni * nt)

                    # ---- segment C: last 1024 cols, nblk=2, mi-outer
                    col_c = OUT - awb
                    wC = w_pool.tile([P, KT, awb], mm_dt, tag="w", name="wC")
                    for k in range(KT):
                        nc.sync.dma_start(wC[:, k], w_d[:, k, col_c:col_c + awb])
                    if tuned:
                        nc.sync.dma_start(bias_sb[:, col_c:col_c + awb],
                                          bias_d[:, col_c:col_c + awb])
                    for mi in range(MT):
                        pss = [ps_pool.tile([P, nt], mybir.dt.float32,
                                            tag="ps", name=f"psC_{mi}_{j}")
                               for j in range(2)]
                        for k in range(KT):
                            for ni in range(2):
                                nc.tensor.matmul(
                                    pss[ni][:],
                                    lhsT=xt_sb[mi][:, k, :],
                                    rhs=wC[:, k, ni * nt:(ni + 1) * nt],
                                    start=(k == 0),
                                    stop=(k == KT - 1),
                                )
                        last = mi == MT - 1
                        for ni in range(2):
                            dq = nc.sync if (last and ni == 0) else None
                            evict(pss[ni], mi, col_c + ni * nt, dma_eng=dq)
    nc.compile()
    names = dict(xt=xt_d.name, w=w_d.name, bias=bias_d.name, out=out_d.name)
    return nc, names


def _get4(cap, reps=1, out_dt_name="bfloat16"):
    key = ("v4", cap, reps, out_dt_name)
    if key not in _cache:
        _cache[key] = _build4(cap, reps, out_dt_name)
    return _cache[key]


def _build_ldw(G, reps=1):
    """LDW-exposure microbench: 2048 independent 512-col bf16 matmuls,
    stationary changes every G matmuls (cycling 16 k-slices of one xt
    tile), 8 rotating PSUM banks, start=stop=True each (no chains). One
    eviction at the end so an output exists. Per-MM floor 213.3ns."""
    import contextlib

    import concourse.mybir as mybir
    import concourse.tile as tile
    from concourse import bacc

    mm_dt = mybir.dt.bfloat16
    NMM = 2048

    nc = bacc.Bacc(None, target_bir_lowering=False, debug=False)
    with tile.TileContext(nc) as tc:
        with tc.tile_pool(name="dram", bufs=1, space="DRAM") as dram:
            xt_d = dram.tile((P, 16, P), mm_dt, kind="ExternalInput")
            w_d = dram.tile((P, NTILE), mm_dt, kind="ExternalInput")
            out_d = dram.tile((P, NTILE), mybir.dt.float32,
                              kind="ExternalOutput")
            with tc.tile_pool(name="res", bufs=1) as res_pool, \
                 tc.tile_pool(name="ev", bufs=1) as o_pool, \
                 tc.tile_pool(name="acc", bufs=8, space="PSUM") as ps_pool:
                xt_sb = res_pool.tile([P, 16, P], mm_dt)
                w_sb = res_pool.tile([P, NTILE], mm_dt)
                nc.gpsimd.dma_start(xt_sb[:], xt_d[:])
                nc.gpsimd.dma_start(w_sb[:], w_d[:])
                loop = tc.For_i(0, reps, 1) if reps > 1 else contextlib.nullcontext()
                with loop:
                    ps = None
                    for i in range(NMM):
                        ps = ps_pool.tile([P, NTILE], mybir.dt.float32)
                        nc.tensor.matmul(
                            ps[:],
                            lhsT=xt_sb[:, (i // G) % 16, :],
                            rhs=w_sb[:],
                            start=True, stop=True,
                        )
                    o_sb = o_pool.tile([P, NTILE], mybir.dt.float32)
                    nc.vector.tensor_copy(out=o_sb[:], in_=ps[:])
                    nc.sync.dma_start(out_d[:], o_sb[:])
    nc.compile()
    return nc, dict(xt=xt_d.name, w=w_d.name, out=out_d.name)


def _get_ldw(G, reps=1):
    key = ("ldw", G, reps)
    if key not in _cache:
        _cache[key] = _build_ldw(G, reps)
    return _cache[key]


def _get3(cap, reps=1, out_dt_name="bfloat16"):
    key = ("v3", cap, reps, out_dt_name)
    if key not in _cache:
        _cache[key] = _build3(cap, reps, out_dt_name)
    return _cache[key]


def kernel(inputs, weight, group_sizes, bias):
    import ml_dtypes

    from concourse.bass_utils import run_bass_kernel_spmd

    M = inputs.shape[0]
    gs = np.asarray(group_sizes, dtype=np.int64)
    # per-token expert id exactly as the reference's jnp.repeat(...,
    # total_repeat_length=M): truncate or pad with the last expert id
    ids = np.repeat(np.arange(E), gs)
    ids = ids[:M] if len(ids) >= M else np.concatenate(
        [ids, np.full(M - len(ids), E - 1)])
    counts = np.bincount(ids, minlength=E)
    starts = np.concatenate([[0], np.cumsum(counts)])[:E]

    cap = max(P, int(-(-counts.max() // P) * P))
    nc, names = _get3(cap)

    x = np.asarray(inputs, dtype=np.float32).astype(ml_dtypes.bfloat16)
    w = np.asarray(weight, dtype=np.float32).astype(ml_dtypes.bfloat16)
    bias_rep = np.ascontiguousarray(np.broadcast_to(
        np.asarray(bias, np.float32).astype(ml_dtypes.bfloat16), (P, OUT)))

    in_maps = []
    for e in range(E):
        xe = x[starts[e]:starts[e] + counts[e]]
        if xe.shape[0] < cap:
            xe = np.concatenate(
                [xe, np.zeros((cap - xe.shape[0], IN), ml_dtypes.bfloat16)])
        # [cap, IN] -> (MT, P, KT, P): xt[mi, p, k, j] = X[mi*P+j, k*P+p]
        xt = np.ascontiguousarray(
            xe.reshape(cap // P, P, IN // P, P).transpose(0, 3, 2, 1))
        # [IN, OUT] -> (P, KT, OUT): wt[p, a, n] = W[a*P+p, n]
        we = np.ascontiguousarray(
            w[e].reshape(IN // P, P, OUT).transpose(1, 0, 2))
        in_maps.append({names["xt"]: xt, names["w"]: we,
                        names["bias"]: bias_rep})

    res = run_bass_kernel_spmd(nc, in_maps, core_ids=list(range(E)))
    out = np.empty((M, OUT), dtype=np.float32)
    for e in range(E):
        oe = res.results[e][names["out"]]          # (P, cap//P, OUT) bf16
        oe = oe.astype(np.float32).transpose(1, 0, 2).reshape(cap, OUT)
        out[starts[e]:starts[e] + counts[e]] = oe[:counts[e]]
    return out



# revision 19
# speedup vs baseline: 1.3597x; 1.2171x over previous
"""MoE grouped-linear (ragged matmul + bias) on 8 TRN2 NeuronCores.

Expert-parallel sharding: core e computes tokens of expert e:
    out_e = X_e[cap, 2048] @ W_e[2048, 8192] + bias
Tokens are pre-sorted by expert (contiguous groups), so the "all-to-all"
is a free host-side slice/concat. No on-device collectives.

Per-core kernel (_build2, the production path): bf16 matmuls (rel err
2.4e-3 on this problem, gate is 2e-2), loop order (n-block of 4, mi, k,
ni-in-block) so each stationary x^T[mi,k] serves 4 consecutive 512-col
matmuls — this amortizes/hides LDWEIGHTS, which at ~107 ns per 128-col
load is NOT hidden when the stationary changes every matmul (the old
fp32r kernel's ~258 ns/MM vs the 213.3 ns = 512row/2.4GHz warm floor).
All 8 PSUM banks cycle through the 4 concurrent accumulation chains;
bias is fused into the PSUM eviction on the vector engine; W streams
once in [128,16,2048] blocks with per-k-slice DMAs; X^T + bias load on
the gpsimd queue.

Measured via reps-slope R=1 vs R=33 (no NTFF under axon): ~435-440
us/core in quiet windows = the 2048x213.3ns tensor-engine floor at the
full 2.4 GHz clock (same-stationary microbench hits 219 ns/MM, proving
the clock and that LDW was the old kernel's gap). Under sustained load
or tenant/thermal contention the package duty-cycles the PE clock
(K=4/8 HAM states -> ~1.8-2.0 GHz effective) and ALL kernel variants
(incl. no-evict / same-stationary) converge to ~540-585 us — that
regime is power-bound, not structure-bound. fp8 e4m3 DoubleRow (the
only >1x mode) fails accuracy: 4.0e-2 single-pass, 2.7e-2 even with a
2-pass hi/lo split of either operand (e4m3's ~2.5% per-element error
x sqrt(K=2048) accumulation); 3-pass passes but is slower than bf16.
nt=1024 moving is rejected by the walrus birverifier (512 max).
"""

import numpy as np

E, IN, OUT = 8, 2048, 8192
P = 128
NTILE = 512

_cache = {}


def _build(cap, dtype_name="float32r", reps=1, mode="full", ntile=None):
    import contextlib

    import concourse.mybir as mybir
    import concourse.tile as tile
    from concourse import bacc

    mm_dt = getattr(mybir.dt, dtype_name)
    nt = ntile or NTILE
    KT = IN // P            # 16 k-tiles
    MT = cap // P           # m-tiles per core
    NT = OUT // nt          # n-tiles

    nc = bacc.Bacc(None, target_bir_lowering=False, debug=False)
    with tile.TileContext(nc) as tc:
        with tc.tile_pool(name="dram", bufs=1, space="DRAM") as dram:
            # xt_d[mi, p, k, j] = X[mi*P + j, k*P + p] — per-mi contiguous
            # 1MB slices so the first matmul group can start after ~1MB of DMA
            xt_d = dram.tile((MT, P, KT, P), mm_dt, kind="ExternalInput")
            w_d = dram.tile((P, KT, OUT), mm_dt, kind="ExternalInput")
            bias_d = dram.tile((P, OUT), mybir.dt.float32, kind="ExternalInput")
            out_d = dram.tile((P, MT, OUT), mybir.dt.float32, kind="ExternalOutput")

            with tc.tile_pool(name="resident", bufs=1) as res_pool, \
                 tc.tile_pool(name="wchunk", bufs=2) as w_pool, \
                 tc.tile_pool(name="evict", bufs=6) as o_pool, \
                 tc.tile_pool(name="acc", bufs=(3 if nt > 512 else 6), space="PSUM") as ps_pool:
                loop = tc.For_i(0, reps, 1) if reps > 1 else contextlib.nullcontext()
                with loop:
                    # W stream owns the sync queue; X^T + bias load in
                    # parallel on the gpsimd queue, first-needed first.
                    w_sbs = [None] * NT
                    w_sbs[0] = w_pool.tile([P, KT, nt], mm_dt, tag="w",
                                           name="w_sb0")
                    nc.sync.dma_start(w_sbs[0][:], w_d[:, :, 0:nt])

                    xt_sb = [res_pool.tile([P, KT, P], mm_dt, tag=f"xt{mi}",
                                           name=f"xt_sb{mi}")
                             for mi in range(MT)]
                    bias_sb = res_pool.tile([P, OUT], mybir.dt.float32)
                    nc.gpsimd.dma_start(xt_sb[0][:], xt_d[0])
                    nc.gpsimd.dma_start(bias_sb[:], bias_d[:])
                    for mi in range(1, MT):
                        nc.gpsimd.dma_start(xt_sb[mi][:], xt_d[mi])

                    for ni in range(NT):
                        w_sb = w_sbs[ni]
                        if w_sb is None and mode in ("mm_only", "same_w"):
                            w_sb = w_sbs[0]
                        elif w_sb is None:
                            w_sb = w_pool.tile([P, KT, nt], mm_dt, tag="w",
                                               name=f"w_sb{ni}")
                            nc.sync.dma_start(
                                w_sb[:], w_d[:, :, ni * nt:(ni + 1) * nt])
                        for mi in range(MT):
                            ps = ps_pool.tile([P, nt], mybir.dt.float32)
                            for k in range(KT):
                                nc.tensor.matmul(
                                    ps[:],
                                    lhsT=xt_sb[0][:, 0, :] if mode == "same_w"
                                    else xt_sb[mi][:, k, :],
                                    rhs=w_sb[:, k, :],
                                    start=(k == 0),
                                    stop=(k == KT - 1),
                                )
                            if mode in ("mm_only", "same_w") and not (ni == NT - 1 and mi == MT - 1):
                                continue
                            o_sb = o_pool.tile([P, nt], mybir.dt.float32)
                            nc.vector.tensor_add(
                                out=o_sb[:], in0=ps[:],
                                in1=bias_sb[:, ni * nt:(ni + 1) * nt])
                            nc.sync.dma_start(
                                out_d[:, mi, ni * nt:(ni + 1) * nt], o_sb[:])
    nc.compile()
    names = dict(xt=xt_d.name, w=w_d.name, bias=bias_d.name, out=out_d.name)
    return nc, names


def _get(cap, dtype_name="float32r", reps=1, mode="full", ntile=None):
    key = (cap, dtype_name, reps, mode, ntile)
    if key not in _cache:
        _cache[key] = _build(cap, dtype_name, reps, mode, ntile)
    return _cache[key]


def _build2(cap, dtype_name="bfloat16", reps=1, nblk=4, psum_bufs=8,
            mode="full"):
    """LDW-amortized variant: loop (ni_blk, mi, k, ni-in-blk) so each
    stationary x^T[mi,k] serves `nblk` consecutive 512-col matmuls.
    W is streamed once, in [128, KT, nblk*512] blocks, per-k-slice DMAs.
    mode: full | no_evict (only last gen evicts) | same_w (fixed stationary)
    """
    import contextlib

    import concourse.mybir as mybir
    import concourse.tile as tile
    from concourse import bacc

    mm_dt = getattr(mybir.dt, dtype_name)
    nt = NTILE                      # 512
    KT = IN // P                    # 16
    MT = cap // P                   # m-tiles
    NBLK = OUT // (nblk * nt)       # blocks of nblk n-tiles
    bw = nblk * nt                  # block width in cols

    nc = bacc.Bacc(None, target_bir_lowering=False, debug=False)
    with tile.TileContext(nc) as tc:
        with tc.tile_pool(name="dram", bufs=1, space="DRAM") as dram:
            xt_d = dram.tile((MT, P, KT, P), mm_dt, kind="ExternalInput")
            w_d = dram.tile((P, KT, OUT), mm_dt, kind="ExternalInput")
            bias_d = dram.tile((P, OUT), mybir.dt.float32, kind="ExternalInput")
            out_d = dram.tile((P, MT, OUT), mybir.dt.float32, kind="ExternalOutput")

            with tc.tile_pool(name="resident", bufs=1) as res_pool, \
                 tc.tile_pool(name="wblk", bufs=2) as w_pool, \
                 tc.tile_pool(name="evict", bufs=4) as o_pool, \
                 tc.tile_pool(name="acc", bufs=psum_bufs, space="PSUM") as ps_pool:
                loop = tc.For_i(0, reps, 1) if reps > 1 else contextlib.nullcontext()
                with loop:
                    w_sbs = [None] * NBLK
                    w_sbs[0] = w_pool.tile([P, KT, bw], mm_dt, tag="w",
                                           name="w_sb0")
                    # per-k-slice DMAs so the first matmul is gated on
                    # one k-slice, not the whole 8MB block
                    for k in range(KT):
                        nc.sync.dma_start(w_sbs[0][:, k], w_d[:, k, 0:bw])

                    xt_sb = [res_pool.tile([P, KT, P], mm_dt, tag=f"xt{mi}",
                                           name=f"xt_sb{mi}")
                             for mi in range(MT)]
                    bias_sb = res_pool.tile([P, OUT], mybir.dt.float32)
                    nc.gpsimd.dma_start(xt_sb[0][:], xt_d[0])
                    nc.gpsimd.dma_start(bias_sb[:], bias_d[:])
                    for mi in range(1, MT):
                        nc.gpsimd.dma_start(xt_sb[mi][:], xt_d[mi])

                    for blk in range(NBLK):
                        w_sb = w_sbs[blk]
                        if w_sb is None:
                            w_sb = w_pool.tile([P, KT, bw], mm_dt, tag="w",
                                               name=f"w_sb{blk}")
                            for k in range(KT):
                                nc.sync.dma_start(
                                    w_sb[:, k],
                                    w_d[:, k, blk * bw:(blk + 1) * bw])
                        for mi in range(MT):
                            pss = [ps_pool.tile([P, nt], mybir.dt.float32,
                                                tag="ps",
                                                name=f"ps{blk}_{mi}_{j}")
                                   for j in range(nblk)]
                            for k in range(KT):
                                for ni in range(nblk):
                                    nc.tensor.matmul(
                                        pss[ni][:],
                                        lhsT=xt_sb[0][:, 0, :] if mode == "same_w"
                                        else xt_sb[mi][:, k, :],
                                        rhs=w_sb[:, k, ni * nt:(ni + 1) * nt],
                                        start=(k == 0),
                                        stop=(k == KT - 1),
                                    )
                            if mode in ("no_evict", "same_w") and not (
                                    blk == NBLK - 1 and mi == MT - 1):
                                continue
                            for ni in range(nblk):
                                o_sb = o_pool.tile([P, nt], mybir.dt.float32)
                                col0 = blk * bw + ni * nt
                                nc.vector.tensor_add(
                                    out=o_sb[:], in0=pss[ni][:],
                                    in1=bias_sb[:, col0:col0 + nt])
                                nc.sync.dma_start(
                                    out_d[:, mi, col0:col0 + nt], o_sb[:])
    nc.compile()
    names = dict(xt=xt_d.name, w=w_d.name, bias=bias_d.name, out=out_d.name)
    return nc, names


def _get2(cap, dtype_name="bfloat16", reps=1, nblk=4, psum_bufs=8,
          mode="full"):
    key = ("v2", cap, dtype_name, reps, nblk, psum_bufs, mode)
    if key not in _cache:
        _cache[key] = _build2(cap, dtype_name, reps, nblk, psum_bufs, mode)
    return _cache[key]


def _build3(cap, reps=1, out_dt_name="bfloat16"):
    """Cold-start-optimized variant.

    Block A (first 2048 cols): k-outer over mi-pairs so the first sweep
    consumes W k-slices at ~300GB/s (supply ~350) instead of mi-outer's
    600GB/s — kills the block-0 DMA stall. G=4 stationary reuse kept
    (each xt[mi,k] serves ni0..3); 2mi x 4ni = 8 live PSUM banks.
    Blocks B1..B3 (cols 2048..8191): mi-outer as _build2 (prefetched).
    Out DMAs ride the Activation queue so they never block the SP
    queue's W prefetch; non-critical loads (xt2b/3b, xt4..7, biasB) are
    emitted on the Act queue BETWEEN eviction DMAs, so the out-DMAs'
    semaphore waits throttle them until the cold-start window is over.
    bias is bf16 [P, OUT]; outputs are stored bf16 (abs err +<=0.011 vs
    gate 0.114). First matmul gated on ~190KB: W k0 in 4 chunks, xt0/1
    split at k0..1. Tail: last mi's 4 out-DMAs split across SP + Act.
    """
    import contextlib

    import concourse.mybir as mybir
    import concourse.tile as tile
    from concourse import bacc

    mm_dt = mybir.dt.bfloat16
    out_dt = getattr(mybir.dt, out_dt_name)
    nt = NTILE                      # 512
    KT = IN // P                    # 16
    MT = cap // P                   # m-tiles
    nblk = 4
    bw = nblk * nt                  # 2048
    NBLK = OUT // bw                # 4 (A + 3 B-blocks)
    GM = min(2, MT)                 # mi-group size in block A
    tuned = MT == 8                 # DMA schedule tuned for cap=1024

    nc = bacc.Bacc(None, target_bir_lowering=False, debug=False)
    with tile.TileContext(nc) as tc:
        with tc.tile_pool(name="dram", bufs=1, space="DRAM") as dram:
            xt_d = dram.tile((MT, P, KT, P), mm_dt, kind="ExternalInput")
            w_d = dram.tile((P, KT, OUT), mm_dt, kind="ExternalInput")
            bias_d = dram.tile((P, OUT), mm_dt, kind="ExternalInput")
            out_d = dram.tile((P, MT, OUT), out_dt, kind="ExternalOutput")

            with tc.tile_pool(name="resident", bufs=1) as res_pool, \
                 tc.tile_pool(name="wblk", bufs=2) as w_pool, \
                 tc.tile_pool(name="evict", bufs=8) as o_pool, \
                 tc.tile_pool(name="acc", bufs=8, space="PSUM") as ps_pool:
                loop = tc.For_i(0, reps, 1) if reps > 1 else contextlib.nullcontext()
                with loop:
                    # ---- block A weights: per-k DMAs, k0 chunked x4,
                    # k1 chunked x2 so the first matmuls gate on 128KB
                    wA = w_pool.tile([P, KT, bw], mm_dt, tag="w", name="wA")
                    for c in range(4):
                        nc.sync.dma_start(wA[:, 0, c * nt:(c + 1) * nt],
                                          w_d[:, 0, c * nt:(c + 1) * nt])
                    for c in range(2):
                        nc.sync.dma_start(
                            wA[:, 1, c * 2 * nt:(c + 1) * 2 * nt],
                            w_d[:, 1, c * 2 * nt:(c + 1) * 2 * nt])
                    for k in range(2, KT):
                        nc.sync.dma_start(wA[:, k], w_d[:, k, 0:bw])

                    # ---- early-critical loads on the gpsimd queue
                    xt_sb = [res_pool.tile([P, KT, P], mm_dt, tag=f"xt{mi}",
                                           name=f"xt_sb{mi}")
                             for mi in range(MT)]
                    bias_sb = res_pool.tile([P, OUT], mm_dt)
                    if tuned:
                        for mi in (0, 1):
                            nc.gpsimd.dma_start(xt_sb[mi][:, 0:2],
                                                xt_d[mi][:, 0:2])
                        for mi in (0, 1):
                            nc.gpsimd.dma_start(xt_sb[mi][:, 2:KT],
                                                xt_d[mi][:, 2:KT])
                        nc.gpsimd.dma_start(bias_sb[:, 0:bw], bias_d[:, 0:bw])
                        # later-needed xt tiles ride the SP queue AFTER
                        # block A's W slices: FIFO keeps them off the
                        # cold-start window; B1's prefetch has 80us slack
                        for mi in range(2, MT):
                            nc.sync.dma_start(xt_sb[mi][:], xt_d[mi])
                    else:
                        for mi in range(MT):
                            nc.gpsimd.dma_start(xt_sb[mi][:], xt_d[mi])
                        nc.gpsimd.dma_start(bias_sb[:], bias_d[:])

                    def evict(ps, mi, col0, dma_eng=None):
                        o_sb = o_pool.tile([P, nt], out_dt)
                        nc.vector.tensor_add(
                            out=o_sb[:], in0=ps[:],
                            in1=bias_sb[:, col0:col0 + nt])
                        (dma_eng or nc.scalar).dma_start(
                            out_d[:, mi, col0:col0 + nt], o_sb[:])

                    # ---- block A: k-outer over mi-groups
                    for g in range(0, MT, GM):
                        gm = min(GM, MT - g)
                        pss = [ps_pool.tile([P, nt], mybir.dt.float32,
                                            tag="ps", name=f"psA{g}_{j}")
                               for j in range(gm * nblk)]
                        for k in range(KT):
                            for mj in range(gm):
                                for ni in range(nblk):
                                    nc.tensor.matmul(
                                        pss[mj * nblk + ni][:],
                                        lhsT=xt_sb[g + mj][:, k, :],
                                        rhs=wA[:, k, ni * nt:(ni + 1) * nt],
                                        start=(k == 0),
                                        stop=(k == KT - 1),
                                    )
                        for mj in range(gm):
                            for ni in range(nblk):
                                evict(pss[mj * nblk + ni], g + mj, ni * nt)
                        # throttled loads: queued on Act behind this group's
                        # out-DMAs, so they transfer only after the cold
                        # window; each arrives well before it is needed


                    # ---- blocks B1..B3: mi-outer (W prefetched)
                    for blk in range(1, NBLK):
                        w_sb = w_pool.tile([P, KT, bw], mm_dt, tag="w",
                                           name=f"wB{blk}")
                        for k in range(KT):
                            nc.sync.dma_start(
                                w_sb[:, k], w_d[:, k, blk * bw:(blk + 1) * bw])
                        if tuned:
                            nc.sync.dma_start(
                                bias_sb[:, blk * bw:(blk + 1) * bw],
                                bias_d[:, blk * bw:(blk + 1) * bw])
                        for mi in range(MT):
                            pss = [ps_pool.tile([P, nt], mybir.dt.float32,
                                                tag="ps",
                                                name=f"ps{blk}_{mi}_{j}")
                                   for j in range(nblk)]
                            for k in range(KT):
                                for ni in range(nblk):
                                    nc.tensor.matmul(
                                        pss[ni][:],
                                        lhsT=xt_sb[mi][:, k, :],
                                        rhs=w_sb[:, k, ni * nt:(ni + 1) * nt],
                                        start=(k == 0),
                                        stop=(k == KT - 1),
                                    )
                            last = blk == NBLK - 1 and mi == MT - 1
                            for ni in range(nblk):
                                dq = nc.sync if (last and ni < 2) else None
                                evict(pss[ni], mi, blk * bw + ni * nt,
                                      dma_eng=dq)
    nc.compile()
    names = dict(xt=xt_d.name, w=w_d.name, bias=bias_d.name, out=out_d.name)
    return nc, names


def _build4(cap, reps=1, out_dt_name="bfloat16"):
    """A/B/C-segment variant (requires LDW hidden at G=2, measured on HW).

    A: cols 0..1023, k-outer, GM=4 mi-group, nblk=2 (G=2): W demand
    ~150GB/s in the cold window, 8.25MB of early DMA vs ~9.6MB capacity.
    B1..B3: cols 1024..7167, bw=2048 nblk=4 mi-outer (prefetched).
    C: cols 7168..8191, bw=1024 nblk=2 mi-outer: last mi drains only 2
    chains -> short tail; its 2 out-DMAs split across SP/Act queues.
    """
    import contextlib

    import concourse.mybir as mybir
    import concourse.tile as tile
    from concourse import bacc

    mm_dt = mybir.dt.bfloat16
    out_dt = getattr(mybir.dt, out_dt_name)
    nt = NTILE                      # 512
    KT = IN // P                    # 16
    MT = cap // P
    tuned = MT == 8

    nc = bacc.Bacc(None, target_bir_lowering=False, debug=False)
    with tile.TileContext(nc) as tc:
        with tc.tile_pool(name="dram", bufs=1, space="DRAM") as dram:
            xt_d = dram.tile((MT, P, KT, P), mm_dt, kind="ExternalInput")
            w_d = dram.tile((P, KT, OUT), mm_dt, kind="ExternalInput")
            bias_d = dram.tile((P, OUT), mm_dt, kind="ExternalInput")
            out_d = dram.tile((P, MT, OUT), out_dt, kind="ExternalOutput")

            with tc.tile_pool(name="resident", bufs=1) as res_pool, \
                 tc.tile_pool(name="wblk", bufs=2) as w_pool, \
                 tc.tile_pool(name="evict", bufs=8) as o_pool, \
                 tc.tile_pool(name="acc", bufs=8, space="PSUM") as ps_pool:
                loop = tc.For_i(0, reps, 1) if reps > 1 else contextlib.nullcontext()
                with loop:
                    # ---- segment A weights: [P, KT, 1024], per-k DMAs,
                    # k0 in two 512-col chunks
                    awb = 2 * nt    # 1024
                    wA = w_pool.tile([P, KT, awb], mm_dt, tag="w", name="wA")
                    for c in range(2):
                        nc.sync.dma_start(wA[:, 0, c * nt:(c + 1) * nt],
                                          w_d[:, 0, c * nt:(c + 1) * nt])
                    for k in range(1, KT):
                        nc.sync.dma_start(wA[:, k], w_d[:, k, 0:awb])

                    xt_sb = [res_pool.tile([P, KT, P], mm_dt, tag=f"xt{mi}",
                                           name=f"xt_sb{mi}")
                             for mi in range(MT)]
                    bias_sb = res_pool.tile([P, OUT], mm_dt)
                    if tuned:
                        # group0 = mi0..3 in 3 k-tiers (just-in-time),
                        # then xt4..7 + biasA (needed from 27.3us)
                        for mi in range(4):
                            nc.gpsimd.dma_start(xt_sb[mi][:, 0:2],
                                                xt_d[mi][:, 0:2])
                        for mi in range(4):
                            nc.gpsimd.dma_start(xt_sb[mi][:, 2:8],
                                                xt_d[mi][:, 2:8])
                        for mi in range(4):
                            nc.gpsimd.dma_start(xt_sb[mi][:, 8:KT],
                                                xt_d[mi][:, 8:KT])
                        for mi in (4, 5):
                            nc.gpsimd.dma_start(xt_sb[mi][:], xt_d[mi])
                        nc.gpsimd.dma_start(bias_sb[:, 0:awb],
                                            bias_d[:, 0:awb])
                        for mi in (6, 7):
                            nc.gpsimd.dma_start(xt_sb[mi][:], xt_d[mi])
                        # remaining bias loads ride SP after each B/C
                        # block's W slices (FIFO keeps them off the cold
                        # window; no semaphore-waits involved)
                    else:
                        for mi in range(MT):
                            nc.gpsimd.dma_start(xt_sb[mi][:], xt_d[mi])
                        nc.gpsimd.dma_start(bias_sb[:], bias_d[:])

                    def evict(ps, mi, col0, dma_eng=None):
                        o_sb = o_pool.tile([P, nt], out_dt)
                        nc.vector.tensor_add(
                            out=o_sb[:], in0=ps[:],
                            in1=bias_sb[:, col0:col0 + nt])
                        (dma_eng or nc.scalar).dma_start(
                            out_d[:, mi, col0:col0 + nt], o_sb[:])

                    # ---- segment A: k-outer, groups of GM=4 mi, nblk=2
                    GM = min(4, MT)
                    for g in range(0, MT, GM):
                        gm = min(GM, MT - g)
                        pss = [ps_pool.tile([P, nt], mybir.dt.float32,
                                            tag="ps", name=f"psA{g}_{j}")
                               for j in range(gm * 2)]
                        for k in range(KT):
                            for mj in range(gm):
                                for ni in range(2):
                                    nc.tensor.matmul(
                                        pss[mj * 2 + ni][:],
                                        lhsT=xt_sb[g + mj][:, k, :],
                                        rhs=wA[:, k, ni * nt:(ni + 1) * nt],
                                        start=(k == 0),
                                        stop=(k == KT - 1),
                                    )
                        for mj in range(gm):
                            for ni in range(2):
                                evict(pss[mj * 2 + ni], g + mj, ni * nt)

                    # ---- segments B: bw=2048, nblk=4, mi-outer
                    nblk = 4
                    bw = nblk * nt
                    nB = (OUT - 2 * awb) // bw      # 3
                    for blk in range(nB):
                        col_b = awb + blk * bw
                        w_sb = w_pool.tile([P, KT, bw], mm_dt, tag="w",
                                           name=f"wB{blk}")
                        for k in range(KT):
                            nc.sync.dma_start(
                                w_sb[:, k], w_d[:, k, col_b:col_b + bw])
                        if tuned:
                            nc.sync.dma_start(bias_sb[:, col_b:col_b + bw],
                                              bias_d[:, col_b:col_b + bw])
                        for mi in range(MT):
                            pss = [ps_pool.tile([P, nt], mybir.dt.float32,
                                                tag="ps",
                                                name=f"ps{blk}_{mi}_{j}")
                                   for j in range(nblk)]
                            for k in range(KT):
                                for ni in range(nblk):
                                    nc.tensor.matmul(
                                        pss[ni][:],
                                        lhsT=xt_sb[mi][:, k, :],
                                        rhs=w_sb[:, k, ni * nt:(ni + 1) * nt],
                                        start=(k == 0),
                                        stop=(k == KT - 1),
                                    )
                            for ni in range(nblk):
                                evict(pss[ni], mi, col_b + ni * nt)

                    # ---- segment C: last 1024 cols, nblk=2, mi-outer
                    col_c = OUT - awb
                    wC = w_pool.tile([P, KT, awb], mm_dt, tag="w", name="wC")
                    for k in range(KT):
                        nc.sync.dma_start(wC[:, k], w_d[:, k, col_c:col_c + awb])
                    if tuned:
                        nc.sync.dma_start(bias_sb[:, col_c:col_c + awb],
                                          bias_d[:, col_c:col_c + awb])
                    for mi in range(MT):
                        last = mi == MT - 1
                        if last:
                            # two sequential single chains: ni0's eviction
                            # hides under ni1's 3.4us of matmuls, leaving a
                            # single evict+DMA in the drain (G=1 is free
                            # for bf16 per HW measurement)
                            for ni in range(2):
                                ps = ps_pool.tile([P, nt], mybir.dt.float32,
                                                  tag="ps",
                                                  name=f"psC_{mi}_{ni}")
                                for k in range(KT):
                                    nc.tensor.matmul(
                                        ps[:],
                                        lhsT=xt_sb[mi][:, k, :],
                                        rhs=wC[:, k, ni * nt:(ni + 1) * nt],
                                        start=(k == 0),
                                        stop=(k == KT - 1),
                                    )
                                evict(ps, mi, col_c + ni * nt,
                                      dma_eng=nc.sync if ni == 0 else None)
                            continue
                        pss = [ps_pool.tile([P, nt], mybir.dt.float32,
                                            tag="ps", name=f"psC_{mi}_{j}")
                               for j in range(2)]
                        for k in range(KT):
                            for ni in range(2):
                                nc.tensor.matmul(
                                    pss[ni][:],
                                    lhsT=xt_sb[mi][:, k, :],
                                    rhs=wC[:, k, ni * nt:(ni + 1) * nt],
                                    start=(k == 0),
                                    stop=(k == KT - 1),
                                )
                        for ni in range(2):
                            evict(pss[ni], mi, col_c + ni * nt)
    nc.compile()
    names = dict(xt=xt_d.name, w=w_d.name, bias=bias_d.name, out=out_d.name)
    return nc, names


def _get4(cap, reps=1, out_dt_name="bfloat16"):
    key = ("v4", cap, reps, out_dt_name)
    if key not in _cache:
        _cache[key] = _build4(cap, reps, out_dt_name)
    return _cache[key]


def _build_ldw(G, reps=1):
    """LDW-exposure microbench: 2048 independent 512-col bf16 matmuls,
    stationary changes every G matmuls (cycling 16 k-slices of one xt
    tile), 8 rotating PSUM banks, start=stop=True each (no chains). One
    eviction at the end so an output exists. Per-MM floor 213.3ns."""
    import contextlib

    import concourse.mybir as mybir
    import concourse.tile as tile
    from concourse import bacc

    mm_dt = mybir.dt.bfloat16
    NMM = 2048

    nc = bacc.Bacc(None, target_bir_lowering=False, debug=False)
    with tile.TileContext(nc) as tc:
        with tc.tile_pool(name="dram", bufs=1, space="DRAM") as dram:
            xt_d = dram.tile((P, 16, P), mm_dt, kind="ExternalInput")
            w_d = dram.tile((P, NTILE), mm_dt, kind="ExternalInput")
            out_d = dram.tile((P, NTILE), mybir.dt.float32,
                              kind="ExternalOutput")
            with tc.tile_pool(name="res", bufs=1) as res_pool, \
                 tc.tile_pool(name="ev", bufs=1) as o_pool, \
                 tc.tile_pool(name="acc", bufs=8, space="PSUM") as ps_pool:
                xt_sb = res_pool.tile([P, 16, P], mm_dt)
                w_sb = res_pool.tile([P, NTILE], mm_dt)
                nc.gpsimd.dma_start(xt_sb[:], xt_d[:])
                nc.gpsimd.dma_start(w_sb[:], w_d[:])
                loop = tc.For_i(0, reps, 1) if reps > 1 else contextlib.nullcontext()
                with loop:
                    ps = None
                    for i in range(NMM):
                        ps = ps_pool.tile([P, NTILE], mybir.dt.float32)
                        nc.tensor.matmul(
                            ps[:],
                            lhsT=xt_sb[:, (i // G) % 16, :],
                            rhs=w_sb[:],
                            start=True, stop=True,
                        )
                    o_sb = o_pool.tile([P, NTILE], mybir.dt.float32)
                    nc.vector.tensor_copy(out=o_sb[:], in_=ps[:])
                    nc.sync.dma_start(out_d[:], o_sb[:])
    nc.compile()
    return nc, dict(xt=xt_d.name, w=w_d.name, out=out_d.name)


def _build_dr(reps=1):
    """DoubleRow throughput microbench: 2048 fp8e4 DoubleRow matmuls,
    lhsT [P,2,128] (2 stationary planes), rhs [P,2,512] (2 moving
    planes), out [128,512]. Per-MM: 107ns if DoubleRow is 4x bf16
    (cost-model claim), 213ns if 2x (docs claim)."""
    import contextlib

    import concourse.mybir as mybir
    import concourse.tile as tile
    from concourse import bacc

    dt8 = mybir.dt.float8e4
    NMM = 2048

    nc = bacc.Bacc(None, target_bir_lowering=False, debug=False)
    with tile.TileContext(nc) as tc:
        with tc.tile_pool(name="dram", bufs=1, space="DRAM") as dram:
            xt_d = dram.tile((P, 2, 16, P), dt8, kind="ExternalInput")
            w_d = dram.tile((P, 2, NTILE), dt8, kind="ExternalInput")
            out_d = dram.tile((P, NTILE), mybir.dt.float32,
                              kind="ExternalOutput")
            with tc.tile_pool(name="res", bufs=1) as res_pool, \
                 tc.tile_pool(name="ev", bufs=1) as o_pool, \
                 tc.tile_pool(name="acc", bufs=8, space="PSUM") as ps_pool:
                xt_sb = res_pool.tile([P, 2, 16, P], dt8)
                w_sb = res_pool.tile([P, 2, NTILE], dt8)
                nc.gpsimd.dma_start(xt_sb[:], xt_d[:])
                nc.gpsimd.dma_start(w_sb[:], w_d[:])
                loop = tc.For_i(0, reps, 1) if reps > 1 else contextlib.nullcontext()
                with loop:
                    ps = None
                    for i in range(NMM):
                        ps = ps_pool.tile([P, NTILE], mybir.dt.float32)
                        nc.tensor.matmul(
                            ps[:],
                            lhsT=xt_sb[:, :, (i // 4) % 16, :],
                            rhs=w_sb[:],
                            start=True, stop=True,
                            perf_mode=mybir.MatmulPerfMode.DoubleRow,
                        )
                    o_sb = o_pool.tile([P, NTILE], mybir.dt.float32)
                    nc.vector.tensor_copy(out=o_sb[:], in_=ps[:])
                    nc.sync.dma_start(out_d[:], o_sb[:])
    nc.compile()
    return nc, dict(xt=xt_d.name, w=w_d.name, out=out_d.name)


def _get_dr(reps=1):
    key = ("dr", reps)
    if key not in _cache:
        _cache[key] = _build_dr(reps)
    return _cache[key]


def _get_ldw(G, reps=1):
    key = ("ldw", G, reps)
    if key not in _cache:
        _cache[key] = _build_ldw(G, reps)
    return _cache[key]


def _get3(cap, reps=1, out_dt_name="bfloat16"):
    key = ("v3", cap, reps, out_dt_name)
    if key not in _cache:
        _cache[key] = _build3(cap, reps, out_dt_name)
    return _cache[key]


def kernel(inputs, weight, group_sizes, bias):
    import ml_dtypes

    from concourse.bass_utils import run_bass_kernel_spmd

    M = inputs.shape[0]
    gs = np.asarray(group_sizes, dtype=np.int64)
    # per-token expert id exactly as the reference's jnp.repeat(...,
    # total_repeat_length=M): truncate or pad with the last expert id
    ids = np.repeat(np.arange(E), gs)
    ids = ids[:M] if len(ids) >= M else np.concatenate(
        [ids, np.full(M - len(ids), E - 1)])
    counts = np.bincount(ids, minlength=E)
    starts = np.concatenate([[0], np.cumsum(counts)])[:E]

    cap = max(P, int(-(-counts.max() // P) * P))
    nc, names = _get4(cap)

    x = np.asarray(inputs, dtype=np.float32).astype(ml_dtypes.bfloat16)
    w = np.asarray(weight, dtype=np.float32).astype(ml_dtypes.bfloat16)
    bias_rep = np.ascontiguousarray(np.broadcast_to(
        np.asarray(bias, np.float32).astype(ml_dtypes.bfloat16), (P, OUT)))

    in_maps = []
    for e in range(E):
        xe = x[starts[e]:starts[e] + counts[e]]
        if xe.shape[0] < cap:
            xe = np.concatenate(
                [xe, np.zeros((cap - xe.shape[0], IN), ml_dtypes.bfloat16)])
        # [cap, IN] -> (MT, P, KT, P): xt[mi, p, k, j] = X[mi*P+j, k*P+p]
        xt = np.ascontiguousarray(
            xe.reshape(cap // P, P, IN // P, P).transpose(0, 3, 2, 1))
        # [IN, OUT] -> (P, KT, OUT): wt[p, a, n] = W[a*P+p, n]
        we = np.ascontiguousarray(
            w[e].reshape(IN // P, P, OUT).transpose(1, 0, 2))
        in_maps.append({names["xt"]: xt, names["w"]: we,
                        names["bias"]: bias_rep})

    res = run_bass_kernel_spmd(nc, in_maps, core_ids=list(range(E)))
    out = np.empty((M, OUT), dtype=np.float32)
    for e in range(E):
        oe = res.results[e][names["out"]]          # (P, cap//P, OUT) bf16
        oe = oe.astype(np.float32).transpose(1, 0, 2).reshape(cap, OUT)
        out[starts[e]:starts[e] + counts[e]] = oe[:counts[e]]
    return out

